# revision 29
# baseline (speedup 1.0000x reference)
"""JacobiKANLinear TRN2 Bass kernel.

out = silu(x) @ W_base^T + einsum('bik,oik->bo', P(tanh(x)), C) + bias

Host-side algebra: Jacobi polynomials (A=B=1, degree 5) are re-expressed in
the monomial basis.  D[o,i,j] = sum_k C[o,i,k] * T[k,j] where T holds the
monomial coefficients of P_k.  The j=0 term is constant (t^0 == 1) and folds
into the bias.  The device computes 6 feature blocks
[silu(x), t, t^2, t^3, t^4, t^5] (t = tanh(x)) and one fused matmul with
contraction 6*1024 = 6144.

Sharding (8 cores): 4 batch groups x 2 out-feature halves.  Per core:
batch shard 2048 rows, out shard 512 cols.

Matmuls run in bf16 (216 ns vs 227 ns per 512-wide matmul for f32r, and
half the weight-load bytes).  To keep quantization error to one rounding
per operand, the t-powers are computed in f32 (t_f, t2_f, t3_f temps) and
rounded once to bf16 (t4/t5 are products of f32 temps with bf16 output).
Accumulation is f32 in PSUM.  Measured rel-err ~1.1e-2 vs the 2e-2 gate.

Schedule: the 6.3 MB weight block streams during a kt-wavefront over the
first P1 chunks (chunk c lags chunk c-1 by LAG kt steps, so a
not-yet-ready activation never blocks ready matmuls in the in-order PE
stream); remaining chunks run chunk-major.  Bias is a precomputed [128,512]
broadcast DMA folded into the PSUM->SBUF copy.  Warm-up matmuls fed from
memset tiles start the PE p-state ramp with no DMA dependency.  x0 rides
the Act hwdge ring, x1-x3 the gpsimd software DGE, weights the SP ring —
three parallel descriptor paths.
"""
import numpy as np
import ml_dtypes

import concourse.bass as bass
import concourse.mybir as mybir
import concourse.tile as tile
from concourse import bacc
from concourse.bass_utils import run_bass_kernel_spmd

BATCH = 8192
IN_F = 1024
OUT_F = 1024
DEGREE = 5
A = 1.0
B = 1.0

N_CORES = 8
BATCH_GROUPS = 4
OUT_HALVES = 2
B_SHARD = BATCH // BATCH_GROUPS        # 2048
O_SHARD = OUT_F // OUT_HALVES          # 512
N_BLOCKS = DEGREE + 1                  # 6 feature blocks
N_KT = N_BLOCKS * IN_F // 128          # 48 contraction tiles of 128
N_CHUNKS = B_SHARD // 128              # 16 batch chunks per core
IT_PER_BLOCK = IN_F // 128             # 8 in-feature tiles per block

P1 = 4                                 # chunks processed kt-major during w load
SETS = 5                               # rotating feature-block tag sets
LAG = 4                                # wavefront lag in kt steps

F32 = mybir.dt.float32
F16 = mybir.dt.float16
BF16 = mybir.dt.bfloat16
SILU = mybir.ActivationFunctionType.Silu
TANH = mybir.ActivationFunctionType.Tanh
COPY = mybir.ActivationFunctionType.Copy


def _jacobi_monomial_matrix():
    """T[k, j] = coefficient of t^j in P_k (A=B=1), float64."""
    T = np.zeros((DEGREE + 1, DEGREE + 1), dtype=np.float64)
    polys = [np.zeros(DEGREE + 1) for _ in range(DEGREE + 1)]
    polys[0][0] = 1.0
    if DEGREE >= 1:
        polys[1][1] = A + 1.0
        polys[1][0] = 0.5 * (A - B)
    for k in range(2, DEGREE + 1):
        alpha_n = 2.0 * k * (k + A + B) * (2 * k + A + B - 2)
        beta_n = (2 * k + A + B - 1) * (A ** 2 - B ** 2)
        gamma_n = (2 * k + A + B - 2) * (2 * k + A + B - 1) * (2 * k + A + B)
        delta_n = 2.0 * (k + A - 1) * (k + B - 1) * (2 * k + A + B)
        p = np.zeros(DEGREE + 1)
        p += (beta_n / gamma_n) * polys[k - 1]
        p[1:] += (alpha_n / gamma_n) * polys[k - 1][:-1]
        p -= (delta_n / gamma_n) * polys[k - 2]
        polys[k] = p
    for k in range(DEGREE + 1):
        T[k] = polys[k]
    return T


def _build_nc():
    nc = bacc.Bacc()
    xt_in = nc.declare_dram_parameter(
        "xt", [N_CHUNKS, 128, IT_PER_BLOCK, 128], F16, isOutput=False)
    w_in = nc.declare_dram_parameter(
        "w", [128, N_KT, O_SHARD], BF16, isOutput=False)
    biasbc_in = nc.declare_dram_parameter(
        "biasbc", [128, O_SHARD], F32, isOutput=False)
    out = nc.declare_dram_parameter("out", [B_SHARD, O_SHARD], F32, isOutput=True)

    with tile.TileContext(nc) as tc:
        with tc.tile_pool(name="wpool", bufs=1) as wpool, \
             tc.tile_pool(name="bpool", bufs=1) as bpool, \
             tc.tile_pool(name="tpool", bufs=4) as tpool, \
             tc.tile_pool(name="xpool", bufs=4) as xpool, \
             tc.tile_pool(name="opool", bufs=3) as opool, \
             tc.tile_pool(name="psum", bufs=1, space="PSUM") as psum_pool:

            w_sb = wpool.tile([128, N_KT, O_SHARD], BF16)
            bias_bc = wpool.tile([128, O_SHARD], F32)
            warm_l = wpool.tile([1, 128], BF16)
            warm_r = wpool.tile([1, O_SHARD], BF16)

            # Descriptor paths: x0 (as two half DMAs, so activations can
            # start on the first half) and x2 ride the Act hwdge ring;
            # x1 (halved) and x3 go through gpsimd software DGE; weights
            # own the SP ring.  Three parallel paths, none deep enough to
            # block its sequencer.  DMA completion semaphores fire per
            # dma_start, so the split is what enables early partial reads.
            half = IT_PER_BLOCK // 2
            x_tiles = []
            for c in range(P1):
                x_c = xpool.tile([128, IT_PER_BLOCK, 128], F16, tag="x",
                                 name=f"x_{c}")
                x_tiles.append(x_c)
            nc.scalar.dma_start(out=x_tiles[0][:, :half, :],
                                in_=xt_in[0][:, :half, :])
            nc.scalar.dma_start(out=x_tiles[0][:, half:, :],
                                in_=xt_in[0][:, half:, :])
            nc.scalar.dma_start(out=x_tiles[2][:], in_=xt_in[2])
            nc.gpsimd.dma_start(out=x_tiles[1][:, :half, :],
                                in_=xt_in[1][:, :half, :])
            nc.gpsimd.dma_start(out=x_tiles[1][:, half:, :],
                                in_=xt_in[1][:, half:, :])
            nc.gpsimd.dma_start(out=x_tiles[3][:], in_=xt_in[3])

            # SP stream: first two kt tiles individually (so kt0 lands as
            # early as possible), then kt pairs; bias broadcast mid-stream
            # (first needed by the chunk-0 epilogue much later).
            nc.sync.dma_start(out=w_sb[:, 0, :], in_=w_in[:, 0, :])
            nc.sync.dma_start(out=w_sb[:, 1, :], in_=w_in[:, 1, :])
            for kp in range(1, N_KT // 2):
                nc.sync.dma_start(
                    out=w_sb[:, 2 * kp:2 * kp + 2, :],
                    in_=w_in[:, 2 * kp:2 * kp + 2, :])
                if kp == 6:
                    nc.sync.dma_start(out=bias_bc[:], in_=biasbc_in[:])

            # PE warm-up fillers fed from memset tiles: no DMA dependency,
            # so the PE goes busy right at context entry and p-state ramps
            # before the first real matmul.
            nc.gpsimd.memset(warm_l[:].bitcast(mybir.dt.uint32), 0)
            nc.gpsimd.memset(warm_r[:].bitcast(mybir.dt.uint32), 0)
            warm_ps = psum_pool.tile([128, O_SHARD], F32, tag="warm")
            for _ in range(3):
                nc.tensor.matmul(
                    warm_ps[:], warm_l[:], warm_r[:], start=True, stop=True)

            def alloc_set(s):
                # bf16 matmul blocks.  t4 later reuses the silu slot, t5
                # the t slot (writes ordered between the slots' consumers).
                a_t = bpool.tile([128, IT_PER_BLOCK, 128], BF16,
                                 tag=f"A{s}", name=f"blkA{s}")
                b_t = bpool.tile([128, IT_PER_BLOCK, 128], BF16,
                                 tag=f"B{s}", name=f"blkB{s}")
                t2b = bpool.tile([128, IT_PER_BLOCK, 128], BF16,
                                 tag=f"C{s}", name=f"blkC{s}")
                t3b = bpool.tile([128, IT_PER_BLOCK, 128], BF16,
                                 tag=f"D{s}", name=f"blkD{s}")
                return a_t, b_t, t2b, t3b

            def alloc_tmp():
                # f32 power-chain temps: one rounding per bf16 block.
                t_f = tpool.tile([128, IT_PER_BLOCK, 128], F32, tag="tf",
                                 name="t_f")
                t2f = tpool.tile([128, IT_PER_BLOCK, 128], F32, tag="t2f",
                                 name="t2f")
                t3f = tpool.tile([128, IT_PER_BLOCK, 128], F32, tag="t3f",
                                 name="t3f")
                return t_f, t2f, t3f

            def emit_powers_dve(blks, tmps):
                _, b_t, t2b, t3b = blks
                t_f, t2f, t3f = tmps
                nc.vector.tensor_mul(t2f[:], t_f[:], t_f[:])
                nc.vector.tensor_copy(t2b[:], t2f[:])
                nc.vector.tensor_mul(t3f[:], t2f[:], t_f[:])
                nc.vector.tensor_copy(t3b[:], t3f[:])

            def finish_chunk(m, ps):
                o_m = opool.tile([128, O_SHARD], F32, tag="o", name=f"o_{m}")
                nc.vector.tensor_add(o_m[:], ps[:], bias_bc[:])
                # The last chunk's output rides the otherwise-idle Act ring
                # so its descriptor generation overlaps the SP drain.
                eng = nc.scalar if m == N_CHUNKS - 1 else nc.sync
                eng.dma_start(out=out[bass.ts(m, 128), :], in_=o_m[:])

            # Phase 1: chunks 0..P1-1 in a kt wavefront.
            blocks1 = [alloc_set(c) for c in range(P1)]
            tmps1 = [alloc_tmp() for c in range(P1)]
            # Act order tuned so each consumer lands just in time: chunk-0
            # silu halves first (matmuls start on half of x0), silu1 before
            # chunk-0's bf16 tanh (c1 joins the wavefront at LAG), then the
            # rest.  The bf16 tanh block is produced directly (a second
            # tanh, not a cast of the f32 one) so it can be scheduled early.
            s0, t0 = blocks1[0][0], blocks1[0][1]
            s1, t1 = blocks1[1][0], blocks1[1][1]
            nc.scalar.activation(s0[:, :half, :], x_tiles[0][:, :half, :], SILU)
            nc.scalar.activation(s0[:, half:, :], x_tiles[0][:, half:, :], SILU)
            nc.scalar.activation(s1[:, :half, :], x_tiles[1][:, :half, :], SILU)
            nc.scalar.activation(t0[:, :half, :], x_tiles[0][:, :half, :], TANH)
            nc.scalar.activation(s1[:, half:, :], x_tiles[1][:, half:, :], SILU)
            nc.scalar.activation(t0[:, half:, :], x_tiles[0][:, half:, :], TANH)
            nc.scalar.activation(blocks1[2][0][:], x_tiles[2][:], SILU)
            nc.scalar.activation(tmps1[0][0][:], x_tiles[0][:], TANH)
            nc.scalar.activation(t1[:, :half, :], x_tiles[1][:, :half, :], TANH)
            nc.scalar.activation(t1[:, half:, :], x_tiles[1][:, half:, :], TANH)
            nc.scalar.activation(blocks1[3][0][:], x_tiles[3][:], SILU)
            nc.scalar.activation(tmps1[1][0][:], x_tiles[1][:], TANH)
            nc.scalar.activation(blocks1[2][1][:], x_tiles[2][:], TANH)
            nc.scalar.activation(tmps1[2][0][:], x_tiles[2][:], TANH)
            nc.scalar.activation(blocks1[3][1][:], x_tiles[3][:], TANH)
            nc.scalar.activation(tmps1[3][0][:], x_tiles[3][:], TANH)
            for c in range(P1):
                emit_powers_dve(blocks1[c], tmps1[c])
            ps1 = [psum_pool.tile([128, O_SHARD], F32, tag="ps", bufs=P1 + 1,
                                  name=f"ps1_{c}") for c in range(P1)]
            for s in range(N_KT + LAG * (P1 - 1)):
                for c in range(P1):
                    kt = s - LAG * c
                    if not 0 <= kt < N_KT:
                        continue
                    a_t, b_t, t2b, t3b = blocks1[c]
                    t_f, t2f, t3f = tmps1[c]
                    b = kt // IT_PER_BLOCK
                    it = kt % IT_PER_BLOCK
                    # t4 overwrites the silu slot once its kt0-7 reads are
                    # emitted; t5 overwrites t after kt8-15.  Both land
                    # well before their first consumer (kt32/kt40).
                    if b == 1 and it == 0:
                        nc.gpsimd.tensor_mul(a_t[:], t2f[:], t2f[:])
                    elif b == 2 and it == 0:
                        nc.gpsimd.tensor_mul(b_t[:], t2f[:], t3f[:])
                    src = (a_t, b_t, t2b, t3b, a_t, b_t)[b]
                    nc.tensor.matmul(
                        ps1[c][:], src[:, it, :], w_sb[:, kt, :],
                        start=(kt == 0), stop=(kt == N_KT - 1))
            # Phase 2: remaining chunks, chunk-major (weights resident).
            # Block prep (x DMA + activations + DVE powers) is emitted two
            # chunks ahead so the DVE never alternates prep with epilogue
            # adds that wait on PSUM stops.
            def prep(m):
                x_m = xpool.tile([128, IT_PER_BLOCK, 128], F16, tag="x",
                                 name=f"x_{m}")
                nc.scalar.dma_start(out=x_m[:], in_=xt_in[m])
                blks = alloc_set(m % SETS)
                tmps = alloc_tmp()
                nc.scalar.activation(blks[0][:], x_m[:], SILU)
                nc.scalar.activation(blks[1][:], x_m[:], TANH)
                nc.scalar.activation(tmps[0][:], x_m[:], TANH)
                emit_powers_dve(blks, tmps)
                return blks, tmps

            prepped = {}
            for m in range(P1, min(P1 + 2, N_CHUNKS)):
                prepped[m] = prep(m)
            for c in range(P1):
                finish_chunk(c, ps1[c])

            for m in range(P1, N_CHUNKS):
                (a_t, b_t, t2b, t3b), (t_f, t2f, t3f) = prepped.pop(m)
                ps = psum_pool.tile([128, O_SHARD], F32, tag="ps", bufs=P1 + 1,
                                    name=f"ps_{m}")
                for b in range(N_BLOCKS):
                    if b == 4:
                        nc.gpsimd.tensor_mul(a_t[:], t2f[:], t2f[:])
                    elif b == 5:
                        nc.gpsimd.tensor_mul(b_t[:], t2f[:], t3f[:])
                    for it in range(IT_PER_BLOCK):
                        kt = b * IT_PER_BLOCK + it
                        src = (a_t, b_t, t2b, t3b, a_t, b_t)[b]
                        nc.tensor.matmul(
                            ps[:], src[:, it, :], w_sb[:, kt, :],
                            start=(kt == 0), stop=(kt == N_KT - 1))
                if m + 2 < N_CHUNKS:
                    prepped[m + 2] = prep(m + 2)
                finish_chunk(m, ps)
    nc.finalize()
    return nc


_NC_CACHE = None


def _get_nc():
    global _NC_CACHE
    if _NC_CACHE is None:
        _NC_CACHE = _build_nc()
    return _NC_CACHE


def _prepare_host(x, base_weight, jacobi_coeffs, bias):
    T = _jacobi_monomial_matrix()
    D = np.einsum("oik,kj->oij", jacobi_coeffs.astype(np.float64), T)
    bias_eff = bias.astype(np.float64) + D[:, :, 0].sum(axis=1)

    # W'[f, o]: 6 blocks of IN_F feature rows: silu -> base_weight, t^j -> D_j
    w_full = np.empty((N_BLOCKS * IN_F, OUT_F), dtype=np.float32)
    w_full[0:IN_F] = base_weight.T
    for j in range(1, N_BLOCKS):
        w_full[j * IN_F:(j + 1) * IN_F] = D[:, :, j].T.astype(np.float32)

    w_halves = []
    bias_halves = []
    for h in range(OUT_HALVES):
        wh = w_full[:, h * O_SHARD:(h + 1) * O_SHARD]
        # SBUF layout [128, N_KT, O_SHARD]: [p, kt, n] = wh[kt*128 + p, n]
        wh = np.ascontiguousarray(
            wh.reshape(N_KT, 128, O_SHARD).transpose(1, 0, 2)
            .astype(ml_dtypes.bfloat16))
        w_halves.append(wh)
        bh = bias_eff[h * O_SHARD:(h + 1) * O_SHARD].astype(np.float32)
        bias_halves.append(
            np.ascontiguousarray(np.broadcast_to(bh[None, :], (128, O_SHARD))))

    xt_groups = []
    for g in range(BATCH_GROUPS):
        xs = x[g * B_SHARD:(g + 1) * B_SHARD]              # (B_SHARD, IN_F)
        # [c, p, it, b] = xs[c*128 + b, it*128 + p]
        xt = np.ascontiguousarray(
            xs.reshape(N_CHUNKS, 128, IT_PER_BLOCK, 128).transpose(0, 3, 2, 1)
            .astype(np.float16))
        xt_groups.append(xt)
    return xt_groups, w_halves, bias_halves


def kernel(x, base_weight, jacobi_coeffs, bias):
    x = np.asarray(x, dtype=np.float32)
    base_weight = np.asarray(base_weight, dtype=np.float32)
    jacobi_coeffs = np.asarray(jacobi_coeffs, dtype=np.float32)
    bias = np.asarray(bias, dtype=np.float32)

    xt_groups, w_halves, bias_halves = _prepare_host(
        x, base_weight, jacobi_coeffs, bias)

    in_maps = []
    for c in range(N_CORES):
        g, h = c // OUT_HALVES, c % OUT_HALVES
        in_maps.append({
            "xt": xt_groups[g],
            "w": w_halves[h],
            "biasbc": bias_halves[h],
        })

    nc = _get_nc()
    res = run_bass_kernel_spmd(nc, in_maps, core_ids=list(range(N_CORES)))

    out = np.empty((BATCH, OUT_F), dtype=np.float32)
    for c in range(N_CORES):
        g, h = c // OUT_HALVES, c % OUT_HALVES
        out[g * B_SHARD:(g + 1) * B_SHARD,
            h * O_SHARD:(h + 1) * O_SHARD] = res.results[c]["out"]
    return out


# revision 31
# speedup vs baseline: 1.0258x; 1.0258x over previous
"""JacobiKANLinear TRN2 Bass kernel.

out = silu(x) @ W_base^T + einsum('bik,oik->bo', P(tanh(x)), C) + bias

Host-side algebra: Jacobi polynomials (A=B=1, degree 5) are re-expressed in
the monomial basis.  D[o,i,j] = sum_k C[o,i,k] * T[k,j] where T holds the
monomial coefficients of P_k.  The j=0 term is constant (t^0 == 1) and folds
into the bias.  The device computes 6 feature blocks
[silu(x), t, t^2, t^3, t^4, t^5] (t = tanh(x)) and one fused matmul with
contraction 6*1024 = 6144.

Sharding (8 cores): 4 batch groups x 2 out-feature halves.  Per core:
batch shard 2048 rows, out shard 512 cols.

Matmuls run in bf16 (216 ns vs 227 ns per 512-wide matmul for f32r, and
half the weight-load bytes).  To keep quantization error to one rounding
per operand, the t-powers are computed in f32 (t_f, t2_f, t3_f temps) and
rounded once to bf16 (t4/t5 are products of f32 temps with bf16 output).
Accumulation is f32 in PSUM.  Measured rel-err ~1.1e-2 vs the 2e-2 gate.

Schedule: the 6.3 MB weight block streams during a kt-wavefront over the
first P1 chunks (chunk c lags chunk c-1 by LAG kt steps, so a
not-yet-ready activation never blocks ready matmuls in the in-order PE
stream); remaining chunks run chunk-major.  Bias is a precomputed [128,512]
broadcast DMA folded into the PSUM->SBUF copy.  Warm-up matmuls fed from
memset tiles start the PE p-state ramp with no DMA dependency.  x0 rides
the Act hwdge ring, x1-x3 the gpsimd software DGE, weights the SP ring —
three parallel descriptor paths.
"""
import numpy as np
import ml_dtypes

import concourse.bass as bass
import concourse.mybir as mybir
import concourse.tile as tile
from concourse import bacc
from concourse.bass_utils import run_bass_kernel_spmd

BATCH = 8192
IN_F = 1024
OUT_F = 1024
DEGREE = 5
A = 1.0
B = 1.0

N_CORES = 8
BATCH_GROUPS = 4
OUT_HALVES = 2
B_SHARD = BATCH // BATCH_GROUPS        # 2048
O_SHARD = OUT_F // OUT_HALVES          # 512
N_BLOCKS = DEGREE + 1                  # 6 feature blocks
N_KT = N_BLOCKS * IN_F // 128          # 48 contraction tiles of 128
N_CHUNKS = B_SHARD // 128              # 16 batch chunks per core
IT_PER_BLOCK = IN_F // 128             # 8 in-feature tiles per block

P1 = 4                                 # chunks processed kt-major during w load
SETS = 5                               # rotating feature-block tag sets
LAG = 4                                # wavefront lag in kt steps

F32 = mybir.dt.float32
F16 = mybir.dt.float16
BF16 = mybir.dt.bfloat16
SILU = mybir.ActivationFunctionType.Silu
TANH = mybir.ActivationFunctionType.Tanh
COPY = mybir.ActivationFunctionType.Copy


def _jacobi_monomial_matrix():
    """T[k, j] = coefficient of t^j in P_k (A=B=1), float64."""
    T = np.zeros((DEGREE + 1, DEGREE + 1), dtype=np.float64)
    polys = [np.zeros(DEGREE + 1) for _ in range(DEGREE + 1)]
    polys[0][0] = 1.0
    if DEGREE >= 1:
        polys[1][1] = A + 1.0
        polys[1][0] = 0.5 * (A - B)
    for k in range(2, DEGREE + 1):
        alpha_n = 2.0 * k * (k + A + B) * (2 * k + A + B - 2)
        beta_n = (2 * k + A + B - 1) * (A ** 2 - B ** 2)
        gamma_n = (2 * k + A + B - 2) * (2 * k + A + B - 1) * (2 * k + A + B)
        delta_n = 2.0 * (k + A - 1) * (k + B - 1) * (2 * k + A + B)
        p = np.zeros(DEGREE + 1)
        p += (beta_n / gamma_n) * polys[k - 1]
        p[1:] += (alpha_n / gamma_n) * polys[k - 1][:-1]
        p -= (delta_n / gamma_n) * polys[k - 2]
        polys[k] = p
    for k in range(DEGREE + 1):
        T[k] = polys[k]
    return T


def _build_nc():
    nc = bacc.Bacc()
    xt_in = nc.declare_dram_parameter(
        "xt", [N_CHUNKS, 128, IT_PER_BLOCK, 128], F16, isOutput=False)
    w_in = nc.declare_dram_parameter(
        "w", [128, N_KT, O_SHARD], BF16, isOutput=False)
    biasbc_in = nc.declare_dram_parameter(
        "biasbc", [128, O_SHARD], F32, isOutput=False)
    out = nc.declare_dram_parameter("out", [B_SHARD, O_SHARD], F32, isOutput=True)

    with tile.TileContext(nc) as tc:
        with tc.tile_pool(name="wpool", bufs=1) as wpool, \
             tc.tile_pool(name="bpool", bufs=1) as bpool, \
             tc.tile_pool(name="tpool", bufs=4) as tpool, \
             tc.tile_pool(name="xpool", bufs=4) as xpool, \
             tc.tile_pool(name="opool", bufs=3) as opool, \
             tc.tile_pool(name="psum", bufs=1, space="PSUM") as psum_pool:

            w_sb = wpool.tile([128, N_KT, O_SHARD], BF16)
            bias_bc = wpool.tile([128, O_SHARD], F32)
            warm_l = wpool.tile([1, 128], BF16)
            warm_r = wpool.tile([1, O_SHARD], BF16)

            # Descriptor paths: x0 (as two half DMAs, so activations can
            # start on the first half) and x2 ride the Act hwdge ring;
            # x1 (halved) and x3 go through gpsimd software DGE; weights
            # own the SP ring.  Three parallel paths, none deep enough to
            # block its sequencer.  DMA completion semaphores fire per
            # dma_start, so the split is what enables early partial reads.
            half = IT_PER_BLOCK // 2
            x_tiles = []
            for c in range(P1):
                x_c = xpool.tile([128, IT_PER_BLOCK, 128], F16, tag="x",
                                 name=f"x_{c}")
                x_tiles.append(x_c)
            nc.scalar.dma_start(out=x_tiles[0][:], in_=xt_in[0])
            nc.scalar.dma_start(out=x_tiles[2][:], in_=xt_in[2])
            nc.gpsimd.dma_start(out=x_tiles[1][:], in_=xt_in[1])
            nc.gpsimd.dma_start(out=x_tiles[3][:], in_=xt_in[3])

            # SP stream: first two kt tiles individually (so kt0 lands as
            # early as possible), then kt pairs; bias broadcast mid-stream
            # (first needed by the chunk-0 epilogue much later).
            nc.sync.dma_start(out=w_sb[:, 0, :], in_=w_in[:, 0, :])
            nc.sync.dma_start(out=w_sb[:, 1, :], in_=w_in[:, 1, :])
            for kp in range(1, N_KT // 2):
                nc.sync.dma_start(
                    out=w_sb[:, 2 * kp:2 * kp + 2, :],
                    in_=w_in[:, 2 * kp:2 * kp + 2, :])
                if kp == 6:
                    nc.sync.dma_start(out=bias_bc[:], in_=biasbc_in[:])

            # PE warm-up fillers fed from memset tiles: no DMA dependency,
            # so the PE goes busy right at context entry and p-state ramps
            # before the first real matmul.
            nc.gpsimd.memset(warm_l[:].bitcast(mybir.dt.uint32), 0)
            nc.gpsimd.memset(warm_r[:].bitcast(mybir.dt.uint32), 0)
            warm_ps = psum_pool.tile([128, O_SHARD], F32, tag="warm")
            for _ in range(3):
                nc.tensor.matmul(
                    warm_ps[:], warm_l[:], warm_r[:], start=True, stop=True)

            def alloc_set(s):
                # bf16 matmul blocks.  t4 later reuses the silu slot, t5
                # the t slot (writes ordered between the slots' consumers).
                a_t = bpool.tile([128, IT_PER_BLOCK, 128], BF16,
                                 tag=f"A{s}", name=f"blkA{s}")
                b_t = bpool.tile([128, IT_PER_BLOCK, 128], BF16,
                                 tag=f"B{s}", name=f"blkB{s}")
                t2b = bpool.tile([128, IT_PER_BLOCK, 128], BF16,
                                 tag=f"C{s}", name=f"blkC{s}")
                t3b = bpool.tile([128, IT_PER_BLOCK, 128], BF16,
                                 tag=f"D{s}", name=f"blkD{s}")
                return a_t, b_t, t2b, t3b

            def alloc_tmp():
                # f32 power-chain temps: one rounding per bf16 block.
                t_f = tpool.tile([128, IT_PER_BLOCK, 128], F32, tag="tf",
                                 name="t_f")
                t2f = tpool.tile([128, IT_PER_BLOCK, 128], F32, tag="t2f",
                                 name="t2f")
                t3f = tpool.tile([128, IT_PER_BLOCK, 128], F32, tag="t3f",
                                 name="t3f")
                return t_f, t2f, t3f

            def emit_powers_dve(blks, tmps):
                _, b_t, t2b, t3b = blks
                t_f, t2f, t3f = tmps
                nc.vector.tensor_mul(t2f[:], t_f[:], t_f[:])
                nc.vector.tensor_copy(t2b[:], t2f[:])
                nc.vector.tensor_mul(t3f[:], t2f[:], t_f[:])
                nc.vector.tensor_copy(t3b[:], t3f[:])

            def finish_chunk(m, ps):
                o_m = opool.tile([128, O_SHARD], F32, tag="o", name=f"o_{m}")
                nc.vector.tensor_add(o_m[:], ps[:], bias_bc[:])
                # The last chunk's output rides the otherwise-idle Act ring
                # so its descriptor generation overlaps the SP drain.
                eng = nc.scalar if m == N_CHUNKS - 1 else nc.sync
                eng.dma_start(out=out[bass.ts(m, 128), :], in_=o_m[:])

            # Phase 1: chunks 0..P1-1 in a kt wavefront.
            blocks1 = [alloc_set(c) for c in range(P1)]
            tmps1 = [alloc_tmp() for c in range(P1)]
            # Act order tuned so each consumer lands just in time: chunk-0
            # silu halves first (matmuls start on half of x0), silu1 before
            # chunk-0's bf16 tanh (c1 joins the wavefront at LAG), then the
            # rest.  The bf16 tanh block is produced directly (a second
            # tanh, not a cast of the f32 one) so it can be scheduled early.
            nc.scalar.activation(blocks1[0][0][:, :half, :],
                                 x_tiles[0][:, :half, :], SILU)
            nc.scalar.activation(blocks1[0][0][:, half:, :],
                                 x_tiles[0][:, half:, :], SILU)
            nc.scalar.activation(blocks1[1][0][:], x_tiles[1][:], SILU)
            nc.scalar.activation(blocks1[0][1][:], x_tiles[0][:], TANH)
            nc.scalar.activation(tmps1[0][0][:], x_tiles[0][:], TANH)
            nc.scalar.activation(blocks1[2][0][:], x_tiles[2][:], SILU)
            nc.scalar.activation(blocks1[1][1][:], x_tiles[1][:], TANH)
            nc.scalar.activation(tmps1[1][0][:], x_tiles[1][:], TANH)
            nc.scalar.activation(blocks1[3][0][:], x_tiles[3][:], SILU)
            nc.scalar.activation(blocks1[2][1][:], x_tiles[2][:], TANH)
            nc.scalar.activation(tmps1[2][0][:], x_tiles[2][:], TANH)
            nc.scalar.activation(blocks1[3][1][:], x_tiles[3][:], TANH)
            nc.scalar.activation(tmps1[3][0][:], x_tiles[3][:], TANH)
            for c in range(P1):
                emit_powers_dve(blocks1[c], tmps1[c])
            ps1 = [psum_pool.tile([128, O_SHARD], F32, tag="ps", bufs=P1 + 1,
                                  name=f"ps1_{c}") for c in range(P1)]
            for s in range(N_KT + LAG * (P1 - 1)):
                for c in range(P1):
                    kt = s - LAG * c
                    if not 0 <= kt < N_KT:
                        continue
                    a_t, b_t, t2b, t3b = blocks1[c]
                    t_f, t2f, t3f = tmps1[c]
                    b = kt // IT_PER_BLOCK
                    it = kt % IT_PER_BLOCK
                    # t4 overwrites the silu slot once its kt0-7 reads are
                    # emitted; t5 overwrites t after kt8-15.  Both land
                    # well before their first consumer (kt32/kt40).
                    if b == 1 and it == 0:
                        nc.gpsimd.tensor_mul(a_t[:], t2f[:], t2f[:])
                    elif b == 2 and it == 0:
                        nc.gpsimd.tensor_mul(b_t[:], t2f[:], t3f[:])
                    src = (a_t, b_t, t2b, t3b, a_t, b_t)[b]
                    nc.tensor.matmul(
                        ps1[c][:], src[:, it, :], w_sb[:, kt, :],
                        start=(kt == 0), stop=(kt == N_KT - 1))
            # Phase 2: remaining chunks, chunk-major (weights resident).
            # Block prep (x DMA + activations + DVE powers) is emitted two
            # chunks ahead so the DVE never alternates prep with epilogue
            # adds that wait on PSUM stops.
            def prep(m):
                x_m = xpool.tile([128, IT_PER_BLOCK, 128], F16, tag="x",
                                 name=f"x_{m}")
                nc.scalar.dma_start(out=x_m[:], in_=xt_in[m])
                blks = alloc_set(m % SETS)
                tmps = alloc_tmp()
                nc.scalar.activation(blks[0][:], x_m[:], SILU)
                nc.scalar.activation(blks[1][:], x_m[:], TANH)
                nc.scalar.activation(tmps[0][:], x_m[:], TANH)
                emit_powers_dve(blks, tmps)
                return blks, tmps

            prepped = {}
            for m in range(P1, min(P1 + 2, N_CHUNKS)):
                prepped[m] = prep(m)
            for c in range(P1):
                finish_chunk(c, ps1[c])

            for m in range(P1, N_CHUNKS):
                (a_t, b_t, t2b, t3b), (t_f, t2f, t3f) = prepped.pop(m)
                ps = psum_pool.tile([128, O_SHARD], F32, tag="ps", bufs=P1 + 1,
                                    name=f"ps_{m}")
                for b in range(N_BLOCKS):
                    if b == 4:
                        nc.gpsimd.tensor_mul(a_t[:], t2f[:], t2f[:])
                    elif b == 5:
                        nc.gpsimd.tensor_mul(b_t[:], t2f[:], t3f[:])
                    for it in range(IT_PER_BLOCK):
                        kt = b * IT_PER_BLOCK + it
                        src = (a_t, b_t, t2b, t3b, a_t, b_t)[b]
                        nc.tensor.matmul(
                            ps[:], src[:, it, :], w_sb[:, kt, :],
                            start=(kt == 0), stop=(kt == N_KT - 1))
                if m + 2 < N_CHUNKS:
                    prepped[m + 2] = prep(m + 2)
                finish_chunk(m, ps)
    nc.finalize()
    return nc


_NC_CACHE = None


def _get_nc():
    global _NC_CACHE
    if _NC_CACHE is None:
        _NC_CACHE = _build_nc()
    return _NC_CACHE


def _prepare_host(x, base_weight, jacobi_coeffs, bias):
    T = _jacobi_monomial_matrix()
    D = np.einsum("oik,kj->oij", jacobi_coeffs.astype(np.float64), T)
    bias_eff = bias.astype(np.float64) + D[:, :, 0].sum(axis=1)

    # W'[f, o]: 6 blocks of IN_F feature rows: silu -> base_weight, t^j -> D_j
    w_full = np.empty((N_BLOCKS * IN_F, OUT_F), dtype=np.float32)
    w_full[0:IN_F] = base_weight.T
    for j in range(1, N_BLOCKS):
        w_full[j * IN_F:(j + 1) * IN_F] = D[:, :, j].T.astype(np.float32)

    w_halves = []
    bias_halves = []
    for h in range(OUT_HALVES):
        wh = w_full[:, h * O_SHARD:(h + 1) * O_SHARD]
        # SBUF layout [128, N_KT, O_SHARD]: [p, kt, n] = wh[kt*128 + p, n]
        wh = np.ascontiguousarray(
            wh.reshape(N_KT, 128, O_SHARD).transpose(1, 0, 2)
            .astype(ml_dtypes.bfloat16))
        w_halves.append(wh)
        bh = bias_eff[h * O_SHARD:(h + 1) * O_SHARD].astype(np.float32)
        bias_halves.append(
            np.ascontiguousarray(np.broadcast_to(bh[None, :], (128, O_SHARD))))

    xt_groups = []
    for g in range(BATCH_GROUPS):
        xs = x[g * B_SHARD:(g + 1) * B_SHARD]              # (B_SHARD, IN_F)
        # [c, p, it, b] = xs[c*128 + b, it*128 + p]
        xt = np.ascontiguousarray(
            xs.reshape(N_CHUNKS, 128, IT_PER_BLOCK, 128).transpose(0, 3, 2, 1)
            .astype(np.float16))
        xt_groups.append(xt)
    return xt_groups, w_halves, bias_halves


def kernel(x, base_weight, jacobi_coeffs, bias):
    x = np.asarray(x, dtype=np.float32)
    base_weight = np.asarray(base_weight, dtype=np.float32)
    jacobi_coeffs = np.asarray(jacobi_coeffs, dtype=np.float32)
    bias = np.asarray(bias, dtype=np.float32)

    xt_groups, w_halves, bias_halves = _prepare_host(
        x, base_weight, jacobi_coeffs, bias)

    in_maps = []
    for c in range(N_CORES):
        g, h = c // OUT_HALVES, c % OUT_HALVES
        in_maps.append({
            "xt": xt_groups[g],
            "w": w_halves[h],
            "biasbc": bias_halves[h],
        })

    nc = _get_nc()
    res = run_bass_kernel_spmd(nc, in_maps, core_ids=list(range(N_CORES)))

    out = np.empty((BATCH, OUT_F), dtype=np.float32)
    for c in range(N_CORES):
        g, h = c // OUT_HALVES, c % OUT_HALVES
        out[g * B_SHARD:(g + 1) * B_SHARD,
            h * O_SHARD:(h + 1) * O_SHARD] = res.results[c]["out"]
    return out
